# revision 26
# baseline (speedup 1.0000x reference)
"""Trainium2 Bass kernel for the VQ commitment-loss problem (fp8 DoubleRow).

Math
----
reference loss = 0.25 * mean((codebook[argmin_k dist] - flat)**2)
               = 0.25/(B*T*D) * sum_n min_k ||flat_n - e_k||^2
since the gathered quantized row realizes exactly the min squared distance.

min_k ||f - e||^2 = ||f||^2 + min_k (||e_k||^2 - 2 f.e_k)

The ||f||^2 term is a tiny O(B*P*T) reduction of the (fp8-rounded) input,
computed on the host via the window-count trick.  The device computes only
the dominant O(N*K*D) term: per core (2 of 16 batches)

  - sum_n min_k (||e_k||^2 - 2 f_n.e_k) via fp8e4 DoubleRow TensorE matmuls
    (256-deep contraction per pass): window tiles [128, 4sub, T] are the
    stationary operand, the codebook scaled by -2 is the moving operand
    (two [128, 4sub, 512] code-half tiles).  ||e_k||^2 rides as three extra
    contraction rows (32*r0 + r1 + r2 fp8 decomposition, precomputed on the
    host, paired with a [32,1,1,1,1] column in the window operand).
  - the [128, 2, 1024] PSUM pair-panels are drained by a split pipeline:
    most pairs are converted f32->fp16 into SBUF by the otherwise-idle
    ScalarE (activation Copy), then min-reduced by VectorE as a fp16
    tensor_tensor min tree running in the 2x_1p DVE mode; a few pairs are
    reduced directly from PSUM in f32 by VectorE so neither engine exceeds
    the TensorE pace of ~1.8us/pair.
  - the per-subtile minima go back to the host, which does the final sum.

DMA choreography: everything flows through the SWDGE (gpsimd) queue — it
spreads packets over all 16 SDMA engines, unlike the dynamic HWDGE rings
which only engage ~3 for this pattern — in FIFO order chosen so pair 0's
operands (first code-half tile, first host-expanded window wave) land as
early as possible.  The host pre-expands the first 1536 window columns of
batch 0 into dense [128, 4, 512] waves (4KB-contiguous per partition, ~2x
the descriptor bandwidth of the strided on-device expansion), and the
matmuls run h-outer so the first code-half can start before the second
lands.  The scalar ring carries the constant ones rows and the final
store, keeping the gpsimd DGE drain off the critical tail.

Host side pads/casts/shards inputs, precomputes the codebook norm rows,
the dense head waves and the ||f||^2 self term, and reduces the per-core
minima.
"""

import numpy as np
import ml_dtypes

B, P, T = 16, 12, 4096
WIN = 41
PAD = (WIN - 1) // 2          # 20
K = 1024
D = P * WIN                   # 492
COMMITMENT_COST = 0.25

NCORES = 8
BC = B // NCORES              # batches per core = 2
TP = T + 2 * PAD              # padded time = 4136
NCHUNK = 4                    # contraction subtiles: 3 pellets * 41 taps = 123 rows
CHROWS = 3 * WIN              # 123
NSUB = BC * T // 128          # 64 subtiles of 128 windows per core
NPAIR = NSUB // 2             # 32 PSUM pair-tiles
NWARM = 13                    # HAM warmup matmuls (bridge PE to main-loop start)
DIRECT_PAIRS = (12, 22, 30)     # pairs drained in f32 by VectorE (ACT catch-up)
NWAVE0 = 3                    # host-expanded 512-col head waves of batch 0
WAVE0W = 512                  # columns per head wave

SCALE = COMMITMENT_COST / (B * T * D)

FP8NP = ml_dtypes.float8_e4m3

_CACHED = {}


def _build_nc():
    import concourse.bacc as bacc
    import concourse.bass as bass
    import concourse.mybir as mybir
    import concourse.tile as tile

    BF = mybir.dt.bfloat16
    F32 = mybir.dt.float32
    F16 = mybir.dt.float16
    F8 = mybir.dt.float8e4
    AX = mybir.AxisListType
    OP = mybir.AluOpType
    ACT = mybir.ActivationFunctionType
    DR = mybir.MatmulPerfMode.DoubleRow

    nc = bacc.Bacc("TRN2", target_bir_lowering=False, debug=False)

    xw_d = nc.dram_tensor("xw", [BC, P, TP], F8, kind="ExternalInput")
    cb_d = nc.dram_tensor("cb", [2, 128, NCHUNK, 512], F8, kind="ExternalInput")
    w0a_d = nc.dram_tensor(
        "w0a", [NWAVE0, 128, NCHUNK, WAVE0W], F8, kind="ExternalInput"
    )
    ones_d = nc.dram_tensor("ones5", [5, NCHUNK * T], F8, kind="ExternalInput")
    out_d = nc.dram_tensor("out", [128, NSUB], F32, kind="ExternalOutput")

    with tile.TileContext(nc) as tc:
        with (
            tc.tile_pool(name="cbpool", bufs=1) as cbpool,
            tc.tile_pool(name="wpool", bufs=1) as wpool,
            tc.tile_pool(name="misc", bufs=1) as misc,
        ):
            # ---- resident codebook code-half tiles [k, subtile, code]; rows
            # 123..125 of subtile 0 carry the host ||e||^2 fp8 decomposition
            cbt = [
                cbpool.tile([128, NCHUNK, 512], F8, tag=f"cb{h}", name=f"cbt{h}")
                for h in range(2)
            ]

            # ---- resident window tiles wt[b]: [128, sub, T] fp8 with
            # wt[b][k, c, t] = xw[b, 3c + k//41, t + k%41] for k < 123;
            # w0a[w]: host-expanded dense waves covering b0 cols [0, 1536)
            wt = [
                wpool.tile([128, NCHUNK, T], F8, tag=f"w{b}", name=f"wt{b}")
                for b in range(BC)
            ]
            w0a = [
                wpool.tile([128, NCHUNK, WAVE0W], F8, tag=f"w0a{w}", name=f"w0a{w}")
                for w in range(NWAVE0)
            ]

            def wslice_dma(c, b, lo, hi):
                nc.gpsimd.dma_start(
                    wt[b][0:CHROWS, c, lo:hi],
                    bass.AP(
                        xw_d,
                        (b * P + 3 * c) * TP + lo,
                        [[TP, 3], [1, WIN], [1, hi - lo]],
                    ),
                )

            # ones rows [32,1,1,1,1] for every window tile on the scalar ring
            for b in range(BC):
                nc.scalar.dma_start(wt[b][CHROWS:128, :, :], ones_d[:])

            # SWDGE FIFO order = landing order: first code-half + first head
            # wave gate pair 0; then the rest of the ladder.
            nc.gpsimd.dma_start(cbt[0][:], cb_d[0])
            nc.gpsimd.dma_start(w0a[0][:], w0a_d[0])
            nc.gpsimd.dma_start(cbt[1][:], cb_d[1])
            for w in range(1, NWAVE0):
                nc.gpsimd.dma_start(w0a[w][:], w0a_d[w])
            W0 = NWAVE0 * WAVE0W
            for lo, hi in ((W0, 2560), (2560, T)):
                for c in range(NCHUNK):
                    wslice_dma(c, 0, lo, hi)
            for lo, hi in ((0, 2048), (2048, T)):
                for c in range(NCHUNK):
                    wslice_dma(c, 1, lo, hi)

            warm_src = misc.tile([128, 512], BF)
            nc.vector.memset(warm_src[:], 0.5)
            mins_buf = misc.tile([128, NSUB], F32)
            lmins = misc.tile([128, 2, 2], F32)

            # ---- HAM warmup: PE busy early so the clock is 2.4 GHz when the
            # real matmuls start.
            with tc.tile_pool(name="pwarm", bufs=1, space="PSUM") as pwarm:
                wps = pwarm.tile([128, 512], F32)
                for _ in range(NWARM):
                    nc.tensor.matmul(
                        wps[:], warm_src[:, 0:128], warm_src[:], start=True, stop=True
                    )

            # ---- main loop: 32 pairs of 128-window subtiles
            with (
                tc.tile_pool(name="pmain", bufs=2, space="PSUM") as pmain,
                tc.tile_pool(name="cvt", bufs=4) as cvt,
                tc.tile_pool(name="drain", bufs=2) as drain,
            ):
                for pair in range(NPAIR):
                    ps = pmain.tile([128, 2, K], F32, tag="ps", name=f"ps_{pair}")
                    for s in range(2):
                        i = pair * 2 + s            # subtile index
                        b = i // (NSUB // BC)
                        toff = (i % (NSUB // BC)) * 128
                        if b == 0 and toff + 128 <= W0:
                            stat = w0a[toff // WAVE0W]
                            soff = toff % WAVE0W
                        else:
                            stat = wt[b]
                            soff = toff
                        for h in range(2):
                            for jp in (0, 2):
                                nc.tensor.matmul(
                                    ps[:, s, 512 * h : 512 * (h + 1)],
                                    stat[:, jp : jp + 2, soff : soff + 128],
                                    cbt[h][:, jp : jp + 2, :],
                                    start=(jp == 0),
                                    stop=(jp == 2),
                                    perf_mode=DR,
                                )
                        if pair == NPAIR - 1:
                            # last pair: per-subtile f32 drain so the final
                            # reduce overlaps the other subtile's matmuls
                            nc.vector.tensor_reduce(
                                mins_buf[:, i : i + 1],
                                ps[:, s, :],
                                axis=AX.X,
                                op=OP.min,
                            )
                    if pair == NPAIR - 1:
                        pass
                    elif pair in DIRECT_PAIRS:
                        # direct f32 drain on VectorE (ACT catch-up slot)
                        nc.vector.tensor_reduce(
                            mins_buf[:, 2 * pair : 2 * pair + 2],
                            ps[:],
                            axis=AX.X,
                            op=OP.min,
                        )
                    else:
                        # ScalarE converts to fp16; VectorE min-tree at 2x
                        sb16 = cvt.tile([128, 2, K], F16, tag="sb16")
                        nc.scalar.activation(sb16[:], ps[:], ACT.Copy)
                        m512 = drain.tile([128, 2, 512], F16, tag="m512")
                        m256 = drain.tile([128, 2, 256], F16, tag="m256")
                        m128 = drain.tile([128, 2, 128], F16, tag="m128")
                        nc.vector.tensor_tensor(
                            m512[:], sb16[:, :, 0:512], sb16[:, :, 512:1024], op=OP.min
                        )
                        nc.vector.tensor_tensor(
                            m256[:], m512[:, :, 0:256], m512[:, :, 256:512], op=OP.min
                        )
                        nc.vector.tensor_tensor(
                            m128[:], m256[:, :, 0:128], m256[:, :, 128:256], op=OP.min
                        )
                        nc.vector.tensor_reduce(
                            mins_buf[:, 2 * pair : 2 * pair + 2],
                            m128[:],
                            axis=AX.X,
                            op=OP.min,
                        )

            # ---- ship the minima; the host does the final sum + scaling
            nc.scalar.dma_start(out_d[:], mins_buf[:])

    nc.compile()
    return nc


def get_nc():
    if "nc" not in _CACHED:
        _CACHED["nc"] = _build_nc()
    return _CACHED["nc"]


def _host_prep(x, codebook):
    """Pad/cast/shard the inputs; returns (per-core in_maps, self term)."""
    x = np.asarray(x, dtype=np.float32)
    codebook = np.asarray(codebook, dtype=np.float32)

    x8 = x.astype(FP8NP)
    xw = np.zeros((B, P, TP), dtype=FP8NP)
    xw[:, :, PAD : PAD + T] = x8

    # value of the fp8-rounded codebook, exactly scaled by -2
    cbb = codebook.astype(FP8NP).astype(np.float32)
    rhs = np.zeros((128, NCHUNK, K), dtype=np.float32)
    for c in range(NCHUNK):
        rhs[:CHROWS, c, :] = -2.0 * cbb[:, CHROWS * c : CHROWS * (c + 1)].T
    rhs8 = rhs.astype(FP8NP)

    # ||e||^2 rows: c = 32*r0 + r1 + r2 in fp8, paired with the [32,1,1,1,1]
    # ones rows of the window tiles
    cnorm = (cbb.astype(np.float64) ** 2).sum(axis=1).astype(np.float32)
    r0 = (cnorm / 32.0).astype(FP8NP)
    rem1 = cnorm - 32.0 * r0.astype(np.float32)
    r1 = rem1.astype(FP8NP)
    rem2 = rem1 - r1.astype(np.float32)
    r2 = rem2.astype(FP8NP)
    rhs8[CHROWS, 0, :] = r0
    rhs8[CHROWS + 1, 0, :] = r1
    rhs8[CHROWS + 2, 0, :] = r2
    # code-half major layout: [h, 128, chunk, 512]
    cb_h = np.ascontiguousarray(
        np.stack([rhs8[:, :, 0:512], rhs8[:, :, 512:1024]])
    )

    ones20 = np.ones((5, NCHUNK * T), dtype=FP8NP)
    ones20[0, :] = FP8NP(32.0)

    # dense im2col head waves of each core's batch 0: w0a[w][k, c, t] =
    # xw[b0, 3c + k//41, 512*w + t + k%41] for k < 123; ones rows below
    W0 = NWAVE0 * WAVE0W
    k_idx = np.arange(CHROWS)
    t_idx = np.arange(W0)
    w0a_all = np.zeros((B, 128, NCHUNK, W0), dtype=FP8NP)
    for c in range(NCHUNK):
        pel = 3 * c + k_idx // WIN                      # [123]
        tap = k_idx % WIN                               # [123]
        w0a_all[:, :CHROWS, c, :] = xw[:, pel[:, None], tap[:, None] + t_idx[None, :]]
    w0a_all[:, CHROWS:, :, :] = 1.0
    w0a_all[:, CHROWS, :, :] = FP8NP(32.0)
    # wave-major: [B, wave, 128, chunk, WAVE0W]
    w0a_w = np.ascontiguousarray(
        w0a_all.reshape(B, 128, NCHUNK, NWAVE0, WAVE0W).transpose(0, 3, 1, 2, 4)
    )

    # host-side ||f||^2 term via the window-count trick
    tau = np.arange(TP, dtype=np.float64)
    cnt = np.minimum(np.minimum(tau + 1.0, float(WIN)), float(TP) - tau)
    xf = xw.astype(np.float64)
    self_term = float((xf * xf * cnt[None, None, :]).sum())

    in_maps = []
    for i in range(NCORES):
        in_maps.append(
            {
                "xw": np.ascontiguousarray(xw[BC * i : BC * (i + 1)]),
                "cb": cb_h,
                "w0a": w0a_w[BC * i],
                "ones5": ones20,
            }
        )
    return in_maps, self_term


def kernel(x, codebook):
    from concourse.bass_utils import run_bass_kernel_spmd

    nc = get_nc()
    in_maps, self_term = _host_prep(x, codebook)
    res = run_bass_kernel_spmd(nc, in_maps, core_ids=list(range(NCORES)))
    total = np.float64(self_term)
    for r in res.results:
        total += r["out"].astype(np.float64).sum()
    return np.array(np.float32(SCALE * total))
